# revision 34
# baseline (speedup 1.0000x reference)
"""GQA attention kernel for Trainium2, 8-core tensor-parallel over kv heads.

Reference computation (fp32):
  q  = query @ q_proj.T + q_bias      -> heads (g-major): dq = gi*H*D + hi*D + d
  kv = query @ kv_proj.T + kv_bias    -> per kv head hi: k = cols [hi*2D, hi*2D+D), v = next D
  attn = softmax(q k^T / sqrt(D));  out = (attn v) @ out_proj.T + out_bias

Sharding: 8 cores; core c handles kv head h0 = c//2 and 4 query-head groups
gis = [0..3] (c even) or [4..7] (c odd). Each core computes a full-shape
partial of the output (rank-256 contribution); host sums the 8 partials.

On-core dataflow (bf16 matmuls, fp32 PSUM accumulation; t = n*L + l):
  P1: QT[dq,t] = qpT.T @ queryT ; KVT[128,t] (k rows 0:64, v rows 64:128)
      KTdup[64:128] <- K (DMA shift); V'[t,65] via PE-transpose of VT + ones col
  P2: per (n, head-pair, 512-l chunk): scores^T via row-tiled matmul pair
      (both heads concurrent in PE quadrants); ONE exp per mc on ACT
      ([128,1024] covering both heads, double-buffered psum so ACT never
      stalls); ones-augmented AV -> [attnout^T; denom]; per (pair,half)
      bulk reciprocal + DRAM-bounce partition broadcast + DVE normalize
  P3: out_partial[t,e] = attnoutT.T @ opT (contract local c, 2 chunks of 128)

The three phases are software-pipelined: P1(n1) and P3(n0) matmuls are
emitted as "filler" units between P2 mc-iterations so the PE works under
the ACT-bound exp stream (ACT = exp only; all PSUM evictions on DVE).
"""
import collections
import sys

sys.path.insert(0, "/opt/trn_rl_repo")

import ml_dtypes
import numpy as np

import concourse.bass as bass
import concourse.mybir as mybir
import concourse.tile as tile
from concourse import bacc

H, G, D = 4, 8, 64
L, N, E = 2048, 2, 2048
T = N * L
P = 128
DQ = 256  # per-core q dim: 4 groups x 64
SCALE = float(D) ** -0.5
F32 = mybir.dt.float32
BF16 = mybir.dt.bfloat16


def pbcast(ap2d, p):
    """[1, F] AP -> [p, F] AP broadcast across partitions (stride 0)."""
    return bass.AP(tensor=ap2d.tensor, offset=ap2d.offset, ap=[[0, p]] + list(ap2d.ap[1:]))


def build_nc():
    nc = bacc.Bacc("TRN2", target_bir_lowering=False, debug=False)
    add = mybir.AluOpType.add

    # host pre-arranges operands p-major in 128-partition blocks so each
    # SBUF load is a single (multi-dim) DMA
    qT = nc.dram_tensor("qT", [P, 16 * T], BF16, kind="ExternalInput").ap()
    qpT = nc.dram_tensor("qpT", [P, 16 * DQ], BF16, kind="ExternalInput").ap()
    kvpT = nc.dram_tensor("kvpT", [P, 16 * P], BF16, kind="ExternalInput").ap()
    opT = nc.dram_tensor("opT", [P, 2 * E], BF16, kind="ExternalInput").ap()
    qb = nc.dram_tensor("qb", [P, 2], F32, kind="ExternalInput").ap()
    kvb = nc.dram_tensor("kvb", [P, 1], F32, kind="ExternalInput").ap()
    ident = nc.dram_tensor("ident", [P, P], BF16, kind="ExternalInput").ap()
    ones16 = nc.dram_tensor("ones16", [P, 16], BF16, kind="ExternalInput").ap()
    out = nc.dram_tensor("out", [T, E], BF16, kind="ExternalOutput").ap()
    denombuf = nc.dram_tensor("denombuf", [1, 8 * 2048], F32, kind="Internal").ap()
    recipbuf = nc.dram_tensor("recipbuf", [1, 8 * 2048], F32, kind="Internal").ap()

    with tile.TileContext(nc) as tc, \
            tc.tile_pool(name="consts", bufs=1) as consts, \
            tc.tile_pool(name="data", bufs=1) as data, \
            tc.tile_pool(name="qtp", bufs=2) as qtp, \
            tc.tile_pool(name="epool", bufs=6) as epool, \
            tc.tile_pool(name="stage", bufs=3) as stage, \
            tc.tile_pool(name="ps", bufs=1, space="PSUM") as ps:
        identb = consts.tile([P, P], BF16)
        nc.scalar.dma_start(out=identb[:], in_=ident)
        qb_sb = consts.tile([P, 2], F32)
        nc.scalar.dma_start(out=qb_sb[:], in_=qb)
        kvb_sb = consts.tile([P, 1], F32)
        nc.scalar.dma_start(out=kvb_sb[:], in_=kvb)

        QT0 = data.tile([P, T], BF16)  # dq 0:128   (gi_loc 0, 1)
        QT1 = data.tile([P, T], BF16)  # dq 128:256 (gi_loc 2, 3)
        KVT = data.tile([P, T], BF16)  # k rows 0:64, v rows 64:128
        KTdup = data.tile([P, T], BF16)  # k rows duplicated at partitions 64:128
        attn0 = data.tile([P, T], BF16)  # attnoutT c-chunk 0 (gi_loc 0, 1)
        attn1 = data.tile([P, T], BF16)  # c-chunk 1 (gi_loc 2, 3)
        Vtmp = data.tile([64, T], BF16)
        Vp = [data.tile([P, 16 * 65], BF16, name=f"vp{n}", tag=f"vp{n}") for n in range(N)]
        opT_sb = data.tile([P, 2 * E], BF16)
        nc.scalar.dma_start(out=opT_sb[:], in_=opT)
        qpT_sb = data.tile([P, 16 * DQ], BF16)
        kvpT_sb = data.tile([P, 16 * P], BF16)
        nc.sync.dma_start(out=kvpT_sb[:], in_=kvpT)
        nc.scalar.dma_start(out=qpT_sb[:], in_=qpT)

        # ---------------- unit builders ----------------
        qtb_ref = {}

        def u_load_qt(n, tcl):
            def run():
                qtb = qtp.tile([P, 16 * 1024], BF16, name=f"qtb{n}{tcl}", tag="qt")
                qtb_ref[(n, tcl)] = qtb
                toff = (2 * n + tcl) * 1024
                for eg in range(4):
                    nc.gpsimd.dma_start(
                        out=qtb.rearrange("p (e t) -> p e t", e=16)[:, 4 * eg:4 * eg + 4],
                        in_=qT.rearrange("p (e t) -> p e t", e=16)
                        [:, 4 * eg:4 * eg + 4, toff:toff + 1024])
            return [run]

        def u_proj_pass(n, tcl, pass_id):
            """pass_id: 0 -> pkv/KVT, 1 -> pq0/QT0, 2 -> pq1/QT1.
            Returns 9 units: 8x (2 ec x 2 lq matmuls... 4 mm each) + evict."""
            st = {}
            toff = (2 * n + tcl) * 1024
            tcols = slice(toff, toff + 1024)

            def w_of(ec):
                if pass_id == 0:
                    return kvpT_sb[:, ec * P:(ec + 1) * P]
                if pass_id == 1:
                    return qpT_sb[:, ec * DQ:ec * DQ + P]
                return qpT_sb[:, ec * DQ + P:(ec + 1) * DQ]

            units = []

            def mk_mm(ecg):
                def run():
                    if "pt" not in st:
                        st["pt"] = ps.tile([P, 1024], F32, name="p1ps",
                                           tag="fill", bufs=1)
                    qtb = qtb_ref[(n, tcl)]
                    for ec in ecg:
                        for lq in range(2):
                            nc.tensor.matmul(
                                st["pt"][:, lq * 512:(lq + 1) * 512], lhsT=w_of(ec),
                                rhs=qtb[:, ec * 1024 + lq * 512:ec * 1024 + (lq + 1) * 512],
                                start=ec == 0, stop=ec == 15)
                return run

            for g in range(16):
                units.append(mk_mm([g]))

            def evict():
                pt = st["pt"]
                if pass_id == 0:
                    nc.vector.tensor_scalar(KVT[:, tcols], pt[:], kvb_sb[:, 0:1],
                                            None, op0=add)
                    nc.sync.dma_start(out=KTdup[64:128, tcols], in_=KVT[0:64, tcols])
                    nc.sync.dma_start(out=Vtmp[0:64, tcols], in_=KVT[64:128, tcols])
                elif pass_id == 1:
                    nc.vector.tensor_scalar(QT0[:, tcols], pt[:], qb_sb[:, 0:1],
                                            None, op0=add)
                else:
                    nc.vector.tensor_scalar(QT1[:, tcols], pt[:], qb_sb[:, 1:2],
                                            None, op0=add)
            units.append(evict)
            return units

        def u_vprime(n, lohalf=True, hihalf=True):
            """ones column + PE transposes building V'[t, 65] for batch n."""
            units = []

            if lohalf:
                def vcol():
                    vc = Vp[n].rearrange("p (m c) -> p m c", c=65)[:, :, 64:65]
                    nc.sync.dma_start(out=vc, in_=ones16)
                units.append(vcol)

            def mk_tr(mcg):
                def run():
                    for mc in mcg:
                        pt = ps.tile([P, 64], BF16, name="ptr", tag="s", bufs=2)
                        nc.tensor.transpose(
                            pt[:], Vtmp[0:64, n * L + mc * P:n * L + (mc + 1) * P],
                            identb[0:64, 0:64])
                        nc.vector.tensor_copy(Vp[n][:, mc * 65:mc * 65 + 64], pt[:])
                return run
            gs = ([0, 1] if lohalf else []) + ([2, 3] if hihalf else [])
            for g in gs:
                units.append(mk_tr(range(4 * g, 4 * g + 4)))
            return units

        def u_p3(n, tt_range, tag):
            """P3 units: per (tt, eo) matmuls + evict + out DMA. tag "s" marks
            tail units (P2 done): rotate over all free psum slots, ACT helps
            with evictions."""
            units = []
            TAILTAGS = [("s", 2), ("avAB", 1), ("fill", 1)]

            def mk(tt, eo, idx):
                trows = slice(tt * P, (tt + 1) * P)
                st = {}

                def run_mm():
                    tg, bf = TAILTAGS[idx % 3] if tag == "s" else (tag, 1)
                    po = st["po"] = ps.tile([P, 1024], F32, name="po", tag=tg, bufs=bf)
                    for cc in range(2):
                        src = attn0 if cc == 0 else attn1
                        for lq in range(2):
                            nc.tensor.matmul(
                                po[:, lq * 512:(lq + 1) * 512], lhsT=src[:, trows],
                                rhs=opT_sb[:, cc * E + eo * 1024 + lq * 512:
                                           cc * E + eo * 1024 + (lq + 1) * 512],
                                start=cc == 0, stop=cc == 1)

                def run_ev():
                    ost = stage.tile([P, 1024], BF16, name="ost", tag="ost", bufs=2)
                    if tag == "s" and eo == 1:
                        nc.scalar.copy(ost[:], st["po"][:])
                    else:
                        nc.vector.tensor_copy(ost[:], st["po"][:])
                    nc.gpsimd.dma_start(out=out[trows, eo * 1024:(eo + 1) * 1024],
                                        in_=ost[:])
                return [run_mm, run_ev]
            idx = 0
            for tt in tt_range:
                for eo in range(2):
                    units.extend(mk(tt, eo, idx))
                    idx += 1
            return units

        # ---------------- filler pump ----------------
        fillers = collections.deque()

        def pump(k=1):
            for _ in range(k):
                if fillers:
                    fillers.popleft()()

        # ---------------- P2 emission ----------------
        def emit_p2(n):
            for pair in range(2):
                QTp = QT0 if pair == 0 else QT1
                attnp = attn0 if pair == 0 else attn1
                for half in range(2):
                    for lcin in range(2):
                        lc = half * 2 + lcin
                        lo = n * L + lc * 512
                        avAB = ps.tile([65, 1024], F32, name="avAB", tag="avAB", bufs=1)

                        def emit_scores(mc):
                            mo = n * L + mc * P
                            s = ps.tile([P, 1024], F32, name="sco", tag="s", bufs=2)
                            nc.tensor.matmul(s[:, 0:512], lhsT=KVT[0:64, mo:mo + P],
                                             rhs=QTp[0:64, lo:lo + 512])
                            nc.tensor.matmul(s[:, 512:1024],
                                             lhsT=KTdup[64:128, mo:mo + P],
                                             rhs=QTp[64:128, lo:lo + 512])
                            e = epool.tile([P, 1024], BF16, name="eab", tag="e")
                            nc.scalar.activation(e[:], s[:],
                                                 mybir.ActivationFunctionType.Exp,
                                                 scale=SCALE)
                            return e

                        def emit_av(mc, e):
                            vw = Vp[n][:, mc * 65:mc * 65 + 65]
                            for lq in range(2):
                                nc.tensor.matmul(avAB[:, lq * 512:(lq + 1) * 512],
                                                 lhsT=vw, rhs=e[:, lq * 512:(lq + 1) * 512],
                                                 start=mc == 0, stop=mc == 15)

                        # scores/exp emitted one mc ahead of AV so the PE
                        # queue never blocks the next exp behind an AV wait.
                        # The very first lc force-pumps kv(tc1)/V' prereqs
                        # of its own mc8-15.
                        boost = n == 0 and pair == 0 and lc == 0
                        e_prev = emit_scores(0)
                        for mc in range(1, 16):
                            e_cur = emit_scores(mc)
                            emit_av(mc - 1, e_prev)
                            e_prev = e_cur
                            if boost:
                                pump(3)
                            elif n == 1:
                                pump(3)
                            else:
                                pump(1)
                        emit_av(15, e_prev)
                        # fast-release eviction: small denom copy + one bulk
                        # cast frees the psum banks; attn copies read staging
                        dn = stage.tile([1, 1024], F32, name="dn", tag="dn")
                        nc.vector.tensor_copy(dn[:], avAB[64:65, :])
                        stgb = stage.tile([64, 1024], BF16, name="stgb", tag="stgb")
                        nc.vector.tensor_copy(stgb[:], avAB[0:64, :])
                        nc.vector.tensor_copy(attnp[0:64, lo:lo + 512], stgb[:, 0:512])
                        nc.sync.dma_start(out=attnp[64:128, lo:lo + 512],
                                          in_=stgb[:, 512:1024])
                        dA = (n * 4 + 2 * pair) * 2048 + lc * 512
                        dB = dA + 2048
                        nc.sync.dma_start(out=denombuf[0:1, dA:dA + 512], in_=dn[:, 0:512])
                        nc.sync.dma_start(out=denombuf[0:1, dB:dB + 512],
                                          in_=dn[:, 512:1024])
                    # normalize this (n, pair, half)
                    seg = (n * 4 + 2 * pair) * 2048
                    hoff = half * 1024
                    packed = stage.tile([P, 16], F32, name="packed", tag="packed")
                    nc.sync.dma_start(
                        out=packed[0:64, :],
                        in_=denombuf[0:1, seg + hoff:seg + hoff + 1024]
                        .rearrange("a (p c) -> (a p) c", p=64))
                    nc.sync.dma_start(
                        out=packed[64:128, :],
                        in_=denombuf[0:1, seg + 2048 + hoff:seg + 2048 + hoff + 1024]
                        .rearrange("a (p c) -> (a p) c", p=64))
                    recp = stage.tile([P, 16], F32, name="recp", tag="recp")
                    nc.vector.reciprocal(recp[:], packed[:])
                    nc.sync.dma_start(
                        out=recipbuf[0:1, seg + hoff:seg + hoff + 1024]
                        .rearrange("a (p c) -> (a p) c", p=64),
                        in_=recp[0:64, :])
                    nc.sync.dma_start(
                        out=recipbuf[0:1, seg + 2048 + hoff:seg + 2048 + hoff + 1024]
                        .rearrange("a (p c) -> (a p) c", p=64),
                        in_=recp[64:128, :])
                    bct = stage.tile([P, 1024], F32, name="bct", tag="bct")
                    nc.sync.dma_start(
                        out=bct[0:64, :],
                        in_=pbcast(recipbuf[0:1, seg + hoff:seg + hoff + 1024], 64))
                    nc.sync.dma_start(
                        out=bct[64:128, :],
                        in_=pbcast(recipbuf[0:1, seg + 2048 + hoff:
                                            seg + 2048 + hoff + 1024], 64))
                    ncols = slice(n * L + hoff, n * L + hoff + 1024)
                    nc.vector.tensor_mul(attnp[:, ncols], attnp[:, ncols], bct[:])
                    # after n1 pair1 half0 is normalized, its P3 rows can go
                    if n == 1 and pair == 1 and half == 0:
                        fillers.extend(u_p3(1, range(16, 24), "fill"))

        # ---------------- schedule ----------------
        # inline head: minimum of P1(n0) for P2(n0) lc0 mc0-7 to start
        for u in u_load_qt(0, 0):
            u()
        for u in u_load_qt(0, 1):
            u()
        # PE pstate warmup on dummy transpose-ident matmuls during the
        # initial qT DMA wait (~12us): keeps the array at full clock when
        # the first real projection matmuls arrive
        wup = ps.tile([P, P], F32, name="wup", tag="fill", bufs=1)
        for _ in range(100):
            nc.tensor.matmul(wup[:], lhsT=identb[:], rhs=identb[:])
        for u in u_proj_pass(0, 0, 0):  # kv tc0
            u()
        for u in u_proj_pass(0, 0, 1):  # q0 tc0
            u()
        for u in u_vprime(0, hihalf=False):  # after q0: off first-exp path
            u()

        # fillers: kv tc1 + V' hi-half FIRST (force-pumped during the first
        # lc so scores mc8-15 have their keys), then rest of P1
        fillers.extend(u_proj_pass(0, 1, 0))  # kv tc1
        fillers.extend(u_vprime(0, lohalf=False))
        fillers.extend(u_proj_pass(0, 1, 1))  # q0 tc1
        fillers.extend(u_proj_pass(0, 0, 2))  # q1 tc0
        fillers.extend(u_proj_pass(0, 1, 2))  # q1 tc1
        fillers.extend(u_load_qt(1, 0))
        fillers.extend(u_proj_pass(1, 0, 0))
        fillers.extend(u_load_qt(1, 1))
        fillers.extend(u_proj_pass(1, 1, 0))
        fillers.extend(u_vprime(1))
        fillers.extend(u_proj_pass(1, 0, 1))
        fillers.extend(u_proj_pass(1, 1, 1))
        fillers.extend(u_proj_pass(1, 0, 2))
        fillers.extend(u_proj_pass(1, 1, 2))

        emit_p2(0)
        fillers.extend(u_p3(0, range(0, 16), "fill"))
        emit_p2(1)
        # drain remaining fillers, then tail P3 (uses freed "s" psum slots)
        while fillers:
            pump(1)
        for u in u_p3(1, range(24, 32), "s"):
            u()

    nc.compile()
    return nc


_NC_CACHE = None


def _get_nc():
    global _NC_CACHE
    if _NC_CACHE is None:
        _NC_CACHE = build_nc()
    return _NC_CACHE


def make_in_maps(query, q_proj, q_bias, kv_proj, kv_bias, out_proj):
    """Host-side sharding. Returns list of 8 per-core input dicts."""
    qT_h = np.ascontiguousarray(
        np.asarray(query, dtype=np.float32).transpose(2, 1, 0).reshape(E, T)
        .reshape(16, P, T).transpose(1, 0, 2).reshape(P, 16 * T)
    ).astype(ml_dtypes.bfloat16)

    def pmajor(a, nb):
        """[nb*128, F] -> [128, nb*F] grouping 128-row blocks along free dim."""
        f = a.shape[1]
        return np.ascontiguousarray(
            a.reshape(nb, P, f).transpose(1, 0, 2).reshape(P, nb * f))
    q_proj = np.asarray(q_proj, dtype=np.float32)
    q_bias = np.asarray(q_bias, dtype=np.float32)
    kv_proj = np.asarray(kv_proj, dtype=np.float32)
    kv_bias = np.asarray(kv_bias, dtype=np.float32)
    out_proj = np.asarray(out_proj, dtype=np.float32)
    ident = np.eye(P, dtype=np.float32)

    in_maps = []
    for c in range(8):
        h0 = c // 2
        gis = range(4) if c % 2 == 0 else range(4, 8)
        rows_q = np.array([gi * (H * D) + h0 * D + d for gi in gis for d in range(D)])
        kv_rows = slice(h0 * 2 * D, (h0 + 1) * 2 * D)
        in_maps.append({
            "qT": qT_h,
            "qpT": pmajor(q_proj[rows_q, :].T, 16).astype(ml_dtypes.bfloat16),
            "kvpT": pmajor(kv_proj[kv_rows, :].T, 16).astype(ml_dtypes.bfloat16),
            "opT": pmajor(out_proj[:, rows_q].T, 2).astype(ml_dtypes.bfloat16),
            "qb": np.ascontiguousarray(q_bias[rows_q].reshape(2, P).T),
            "kvb": np.ascontiguousarray(kv_bias[kv_rows].reshape(P, 1)),
            "ident": ident.astype(ml_dtypes.bfloat16),
            "ones16": np.ones((P, 16), dtype=ml_dtypes.bfloat16),
        })
    return in_maps


def kernel(query, q_proj, q_bias, kv_proj, kv_bias, out_proj, out_bias):
    from concourse.bass_utils import run_bass_kernel_spmd

    nc = _get_nc()
    in_maps = make_in_maps(query, q_proj, q_bias, kv_proj, kv_bias, out_proj)
    res = run_bass_kernel_spmd(nc, in_maps, core_ids=list(range(8)))
    total = np.zeros((T, E), dtype=np.float64)
    for rmap in res.results:
        total += rmap["out"].astype(np.float64)
    total += np.asarray(out_bias, dtype=np.float64)[None, :]
    return np.ascontiguousarray(
        total.reshape(N, L, E).transpose(1, 0, 2)).astype(np.float32)
